# revision 1
# baseline (speedup 1.0000x reference)
"""Causal self-attention (B=2, T=2048, D=1024, NH=16) on 8 Trainium2 NeuronCores.

Sharding: core c handles batch b = c // 4 and heads [4*(c%4), 4*(c%4)+4)
(data parallel over B, head/tensor parallel over NH). Wqkv is column-sliced,
Wproj row-sliced per core; each core emits its partial projection
out_c = Y_c @ Wproj_c (transposed, [D, T]); the host sums the four partials
per batch and adds bproj. No cross-core collectives are needed.

On-device layout: everything keeps the contraction dim on SBUF partitions.
The host passes x^T so QKV^T = W^T @ x^T comes out directly in [dim, token]
layout. Attention is computed as S^T (k on partitions, q on free), which
a) makes Q^T/K^T the natural matmul operands, b) lets the PV matmul consume
exp(S^T) straight from SBUF, and c) yields the softmax denominator for free
via a ones-column appended to V (out row = sum_k P^T[k, q]). Causality is
handled by skipping all-masked (k > q) blocks and masking the diagonal
128x128 block with a precomputed 0/1 triangular mask.

Matmuls run in float32r (TF32-like fast fp32 mode; ~1e-3 max rel err).
Attention accumulates per 1024-token q-half with double-buffered PSUM so the
next half's PV matmuls overlap the previous half's softmax normalization.
"""

import contextlib

import numpy as np

import concourse.bass as bass
import concourse.mybir as mybir
import concourse.tile as tile
from concourse import bacc
from concourse.bass_utils import run_bass_kernel_spmd

B, T, D = 2, 2048, 1024
NH, HD = 16, 64
NCORES = 8
HPC = 4                 # heads per core
HDIM = HPC * HD         # 256 qkv dims per core
KCH = D // 128          # 8 contraction chunks
HQ = T // 2             # q-half length (1024)
F32 = mybir.dt.float32
F32R = mybir.dt.float32r
EXP = mybir.ActivationFunctionType.Exp
MULT = mybir.AluOpType.mult

_NC = None


def _spans(lo, hi):
    """q-spans [s, s+w) covering [lo, hi), each ending on a 512 boundary."""
    out = []
    s = lo
    while s < hi:
        w = min(512 - (s % 512), hi - s)
        out.append((s, w))
        s += w
    return out


def _build(debug=False, reps=1):
    nc = bacc.Bacc()
    xT = nc.declare_dram_parameter("xT", [D, T], F32R, isOutput=False)
    Wa = nc.declare_dram_parameter("Wa", [D, 3 * HDIM], F32R, isOutput=False)
    Wp = nc.declare_dram_parameter("Wp", [HDIM, D], F32R, isOutput=False)
    bqk = nc.declare_dram_parameter("bqk", [128, 4], F32, isOutput=False)
    bv = nc.declare_dram_parameter("bv", [1, HDIM], F32R, isOutput=False)
    tri = nc.declare_dram_parameter("tri", [128, 128], F32R, isOutput=False)
    out = nc.declare_dram_parameter("out", [D, T], F32, isOutput=True)
    if debug:
        dbg_qkT = nc.declare_dram_parameter("dbg_qkT", [128, 4 * T], F32, isOutput=True)
        dbg_vsb = nc.declare_dram_parameter("dbg_vsb", [128, 16 * 512], F32, isOutput=True)
        dbg_yT = nc.declare_dram_parameter("dbg_yT", [128, 2 * T], F32, isOutput=True)
        dbg_rden = nc.declare_dram_parameter("dbg_rden", [HPC, T], F32, isOutput=True)

    with tile.TileContext(nc) as tc:
        with (
            tc.tile_pool(name="persist", bufs=1) as pp,
            tc.tile_pool(name="psB", bufs=2, space="PSUM") as psB,
            tc.tile_pool(name="psO", bufs=2, space="PSUM") as psO,
        ):
            # [partition, chunk, token]: chunks 0-1 = Q^T, 2-3 = K^T
            qkT = pp.tile([128, 4, T], F32R)
            # V1 blocks per (t-chunk, head): even head [V|1@64|0], odd [0|1@32|V@64]
            vsb = pp.tile([128, 16, 4 * 128], F32R)
            yT = pp.tile([128, 2, T], F32R)
            wp = pp.tile([128, 2, D], F32R)
            trisb = pp.tile([128, 128], F32R)
            bqksb = pp.tile([128, 4], F32)
            bvsb = pp.tile([1, HDIM], F32R)
            onesr = pp.tile([1, T], F32R)

            nc.sync.dma_start(trisb[:], tri[:])
            nc.sync.dma_start(bqksb[:], bqk[:])
            nc.sync.dma_start(bvsb[:], bv[:])
            for c in range(2):
                nc.sync.dma_start(wp[:, c, :], Wp[c * 128 : (c + 1) * 128, :])
            nc.gpsimd.memset(onesr[:].bitcast(F32), 1.0)
            nc.gpsimd.memset(vsb[:].bitcast(F32), 0.0)
            # ones columns: even heads at 128j+64 (j=0,2), odd at 128j+32 (j=1,3)
            # (denominator rows land on 32-aligned partitions 64 / 32)
            nc.gpsimd.memset(vsb[:, :, 64::256].bitcast(F32), 1.0)
            nc.gpsimd.memset(vsb[:, :, 160::256].bitcast(F32), 1.0)

            loop_ctx = tc.For_i(0, reps, 1) if reps > 1 else contextlib.nullcontext()
            with loop_ctx, contextlib.ExitStack() as _phase_stack:
                # ---- Phase A: QKV projections ----
                inA = _phase_stack.enter_context(tc.tile_pool(name="inA", bufs=1))
                xs, ws = [], []
                for k in range(KCH):
                    xt = inA.tile([128, T], F32R, tag=f"x{k}")
                    nc.sync.dma_start(xt[:], xT[k * 128 : (k + 1) * 128, :])
                    xs.append(xt)
                    wt = inA.tile([128, 3 * HDIM], F32R, tag=f"w{k}")
                    nc.sync.dma_start(wt[:], Wa[k * 128 : (k + 1) * 128, :])
                    ws.append(wt)

                # Q^T, K^T: 4 chunks of 128 dims x 4 spans of 512 tokens.
                # k-order rotated per block so early matmuls only need the
                # first-arrived x/W chunks (hides the input DMA ramp).
                for sp in range(4):
                    slots = [
                        psB.tile([128, 1024], F32, tag="s", name=f"accqk{h}")
                        for h in range(2)
                    ]
                    accs = [slots[m // 2][:, (m % 2) * 512 : (m % 2) * 512 + 512]
                            for m in range(4)]
                    for k in range(KCH):
                        for m in range(4):
                            nc.tensor.matmul(
                                accs[m],
                                ws[k][:, m * 128 : (m + 1) * 128],
                                xs[k][:, sp * 512 : (sp + 1) * 512],
                                start=(k == 0),
                                stop=(k == KCH - 1),
                            )
                    # bias folded per-partition during PSUM->SBUF copyback
                    for m in range(4):
                        nc.vector.tensor_scalar_add(
                            qkT[:, m, sp * 512 : (sp + 1) * 512],
                            accs[m],
                            bqksb[:, m : m + 1],
                        )

                # V: [token, dim] layout; bias via rank-1 ones x bv matmul
                for t in range(16):
                    acc = psB.tile([128, 1024], F32, tag="s", name="accv")[:, 0:HDIM]
                    ks = [(t + i) % KCH for i in range(KCH)]
                    for i, k in enumerate(ks):
                        nc.tensor.matmul(
                            acc[:],
                            xs[k][:, t * 128 : (t + 1) * 128],
                            ws[k][:, 2 * HDIM : 3 * HDIM],
                            start=(i == 0),
                            stop=False,
                        )
                    nc.tensor.matmul(
                        acc[:],
                        onesr[0:1, t * 128 : (t + 1) * 128],
                        bvsb[0:1, :],
                        start=False,
                        stop=True,
                    )
                    # scatter the 4 heads into their V1 blocks (2 strided copies)
                    src_e = acc[:, 0:192].rearrange("p (h d) -> p h d", d=64)[:, ::2]
                    dst_e = vsb[:, t, 0:320].rearrange("p (h d) -> p h d", d=64)[:, ::4]
                    nc.scalar.copy(dst_e, src_e)
                    src_o = acc[:, 64:256].rearrange("p (h d) -> p h d", d=64)[:, ::2]
                    dst_o = vsb[:, t, 192:512].rearrange("p (h d) -> p h d", d=64)[:, ::4]
                    nc.scalar.copy(dst_o, src_o)

                # ---- Phases B+C: attention per (head, q-half) ----
                _phase_stack.close()
                with (
                    tc.tile_pool(name="wk", bufs=3) as wk,
                    tc.tile_pool(name="nm", bufs=2) as nm,
                    tc.tile_pool(name="ob", bufs=2) as obp,
                ):
                    for j in range(HPC):
                        po = 64 * (j % 2)
                        qc, kc = j // 2, 2 + j // 2
                        dp = 64 if j % 2 == 0 else 32
                        for half in range(2):
                            h0 = half * HQ
                            oh = psO.tile([128, HQ], F32, tag="o")
                            ki_hi = 8 if half == 0 else 16
                            for ki in range(ki_hi):
                                k0 = 128 * ki
                                spl = _spans(max(k0, h0), h0 + HQ)
                                groups = [spl[i : i + 2] for i in range(0, len(spl), 2)]
                                for grp in groups:
                                    # group of 1-2 spans shares one 2-bank PSUM
                                    # tile and a single exp over both
                                    sps = psB.tile([128, 1024], F32, tag="s")
                                    for gi, (s, w) in enumerate(grp):
                                        nc.tensor.matmul(
                                            sps[:, gi * 512 : gi * 512 + w],
                                            qkT[po : po + 64, kc, k0 : k0 + 128],
                                            qkT[po : po + 64, qc, s : s + w],
                                            start=True,
                                            stop=True,
                                        )
                                    ew = (len(grp) - 1) * 512 + grp[-1][1]
                                    psb = wk.tile([128, 1024], F32R, tag="p")
                                    nc.scalar.activation(
                                        psb[:, :ew], sps[:, :ew], EXP, scale=0.125
                                    )
                                    if grp[0][0] == k0:
                                        nc.vector.tensor_tensor(
                                            psb[:, 0:128], psb[:, 0:128], trisb[:], MULT
                                        )
                                    for gi, (s, w) in enumerate(grp):
                                        nc.tensor.matmul(
                                            oh[:, s - h0 : s - h0 + w],
                                            vsb[:, ki, 128 * j : 128 * (j + 1)],
                                            psb[:, gi * 512 : gi * 512 + w],
                                            start=(ki == 0),
                                            stop=(ki == ki_hi - 1),
                                            skip_group_check=True,
                                        )
                            # normalize: O^T[d, q] * (1/denom[q]); the denom row
                            # came free from the ones column of V1
                            drec = nm.tile([1, HQ], F32, tag="dc")
                            nc.vector.reciprocal(drec[0:1, :], oh[dp : dp + 1, :])
                            if debug:
                                nc.sync.dma_start(
                                    dbg_rden[j : j + 1, h0 : h0 + HQ], drec[0:1, :]
                                )
                            rb = nm.tile([128, HQ], F32R, tag="rb")
                            nc.gpsimd.partition_broadcast(
                                rb[:], drec[0:1, :].bitcast(F32R)
                            )
                            nc.vector.tensor_tensor(
                                yT[po : po + 64, qc, h0 : h0 + HQ],
                                oh[po : po + 64, :],
                                rb[po : po + 64, :],
                                MULT,
                            )

                    if debug:
                        nc.sync.dma_start(dbg_qkT[:], qkT[:].bitcast(F32))
                        nc.sync.dma_start(dbg_vsb[:], vsb[:].bitcast(F32))
                        nc.sync.dma_start(dbg_yT[:], yT[:].bitcast(F32))

                    # ---- Phase D: output projection ----
                    # copybacks alternate DVE/ACT; one contiguous 1 MB DMA per
                    # 128-row output block
                    for m in range(8):
                        obt = obp.tile([128, T], F32, tag="obt")
                        for sp in range(4):
                            acc = psB.tile([128, 1024], F32, tag="s", name="accd")[:, 0:512]
                            for c in range(2):
                                nc.tensor.matmul(
                                    acc[:],
                                    wp[:, c, m * 128 : (m + 1) * 128],
                                    yT[:, c, sp * 512 : (sp + 1) * 512],
                                    start=(c == 0),
                                    stop=(c == 1),
                                )
                            dst = obt[:, sp * 512 : (sp + 1) * 512]
                            if sp % 2 == 0:
                                nc.vector.tensor_copy(dst, acc[:])
                            else:
                                nc.scalar.copy(dst, acc[:])
                        nc.sync.dma_start(out[m * 128 : (m + 1) * 128, :], obt[:])

    nc.compile()
    return nc


def _get_nc():
    global _NC
    if _NC is None:
        _NC = _build()
    return _NC


def _make_in_maps(x, Wqkv, bqkv, Wproj):
    x = np.ascontiguousarray(np.asarray(x, np.float32))
    Wqkv = np.asarray(Wqkv, np.float32)
    bqkv = np.asarray(bqkv, np.float32)
    Wproj = np.asarray(Wproj, np.float32)
    tri = np.triu(np.ones((128, 128), np.float32))  # tri[k, q] = q >= k
    in_maps = []
    for c in range(NCORES):
        b = c // 4
        cs = (c % 4) * HDIM
        ce = cs + HDIM
        Wa = np.concatenate(
            [Wqkv[:, cs:ce], Wqkv[:, D + cs : D + ce], Wqkv[:, 2 * D + cs : 2 * D + ce]],
            axis=1,
        )
        bqk_c = np.concatenate([bqkv[cs:ce], bqkv[D + cs : D + ce]])
        in_maps.append(
            {
                "xT": np.ascontiguousarray(x[b].T),
                "Wa": np.ascontiguousarray(Wa),
                "Wp": np.ascontiguousarray(Wproj[cs:ce, :]),
                "bqk": np.ascontiguousarray(bqk_c.reshape(4, 128).T),
                "bv": np.ascontiguousarray(bqkv[2 * D + cs : 2 * D + ce].reshape(1, HDIM)),
                "tri": tri,
            }
        )
    return in_maps


def _run(in_maps, **kwargs):
    nc = _get_nc()
    return run_bass_kernel_spmd(nc, in_maps, core_ids=list(range(NCORES)), **kwargs)


def kernel(x, Wqkv, bqkv, Wproj, bproj):
    in_maps = _make_in_maps(x, Wqkv, bqkv, Wproj)
    res = _run(in_maps)
    bproj = np.asarray(bproj, np.float32)
    outp = np.zeros((B, T, D), np.float32)
    for c in range(NCORES):
        outp[c // 4] += res.results[c]["out"].T
    outp += bproj[None, None, :]
    return outp



# revision 42
# speedup vs baseline: 1.7239x; 1.7239x over previous
"""Causal self-attention (B=2, T=2048, D=1024, NH=16) on 8 Trainium2 NeuronCores.

Sharding: core c handles batch b = c // 4 and heads [4*(c%4), 4*(c%4)+4)
(data parallel over B, head/tensor parallel over NH). Wqkv is column-sliced,
Wproj row-sliced per core; each core emits its partial projection
out_c = Y_c @ Wproj_c (transposed, [D, T]); the host sums the four partials
per batch and adds bproj. No cross-core collectives are needed.

Schedule (HW-calibrated): the ACT engine's exp (~750ns/instr + 0.34ns/col)
and the PE matmul stream are the joint bottleneck, so the kernel is built
as one long software-pipelined stream:
  1. During the input DMA (~37us), Q/K for heads 0-1 (chunks m=0,2) are
     accumulated chunk-by-chunk over the full sequence (4x PSUM [128,2048]
     spread over 2 pools = all 8 banks).
  2. Attention runs half-outer (q in [0,1024) for all heads, then
     [1024,2048)), heads inner. Within a head the S->exp->(tri)->PV chain
     is pipelined with depth 2: PV(g-1) issues after S(g), so the exp of
     group g-1 hides under the S matmul of group g.
  3. Remaining PE work (Q/K for heads 2-3, the V projection, and the
     output projection) is emitted as filler units between attention
     groups: attention is ACT-paced, so fillers use PE idle slots.
  4. The output projection is split per q-half: half-0 projection+DMA
     overlap half-1 attention.

Matmuls run in float32r; all per-(head,half,group) math is bit-identical
to the original reference kernel (same span/group structure, same
accumulation orders).
"""

import contextlib

import numpy as np

import concourse.bass as bass
import concourse.mybir as mybir
import concourse.tile as tile
from concourse import bacc
from concourse.bass_utils import run_bass_kernel_spmd

B, T, D = 2, 2048, 1024
NH, HD = 16, 64
NCORES = 8
HPC = 4                 # heads per core
HDIM = HPC * HD         # 256 qkv dims per core
KCH = D // 128          # 8 contraction chunks
HQ = T // 2             # q-half length (1024)
F32 = mybir.dt.float32
F32R = mybir.dt.float32r
EXP = mybir.ActivationFunctionType.Exp
MULT = mybir.AluOpType.mult

_NC = None


def _spans(lo, hi):
    """q-spans [s, s+w) covering [lo, hi), each ending on a 512 boundary."""
    out = []
    s = lo
    while s < hi:
        w = min(512 - (s % 512), hi - s)
        out.append((s, w))
        s += w
    return out


def _groups(h0, hq, ki_hi):
    """(ki, grp, masked) list for one q-half: grp = 1-2 spans sharing one
    PSUM tile + one exp; masked = group starts at the diagonal block."""
    out = []
    for ki in range(ki_hi):
        k0 = 128 * ki
        spl = _spans(max(k0, h0), h0 + hq)
        for i in range(0, len(spl), 2):
            grp = spl[i : i + 2]
            out.append((ki, grp, grp[0][0] == k0))
    return out


def _build(debug=False, reps=1, phases="iABDo"):
    """phases (ablation/bench only): i=input DMA, A=QKV matmuls,
    B=attention, D=proj matmuls, o=output DMA. Default = full kernel."""
    nc = bacc.Bacc()
    xT = nc.declare_dram_parameter("xT", [D, T], F32R, isOutput=False)
    Wa = nc.declare_dram_parameter("Wa", [D, 3 * HDIM], F32R, isOutput=False)
    Wp = nc.declare_dram_parameter("Wp", [HDIM, D], F32R, isOutput=False)
    bqk = nc.declare_dram_parameter("bqk", [128, 4], F32, isOutput=False)
    bv = nc.declare_dram_parameter("bv", [1, HDIM], F32R, isOutput=False)
    tri = nc.declare_dram_parameter("tri", [128, 128], F32R, isOutput=False)
    out = nc.declare_dram_parameter("out", [D, T], F32, isOutput=True)
    if debug:
        dbg_qkT = nc.declare_dram_parameter("dbg_qkT", [128, 4 * T], F32, isOutput=True)
        dbg_vsb = nc.declare_dram_parameter("dbg_vsb", [128, 16 * 512], F32, isOutput=True)
        dbg_yT = nc.declare_dram_parameter("dbg_yT", [128, 2 * T], F32, isOutput=True)

    with tile.TileContext(nc) as tc:
        with (
            tc.tile_pool(name="persist", bufs=1) as pp,
            tc.tile_pool(name="psS", bufs=2, space="PSUM") as psS,
            tc.tile_pool(name="psO", bufs=2, space="PSUM") as psO,
            tc.tile_pool(name="inA", bufs=1) as inA,
            tc.tile_pool(name="wk", bufs=2) as wk,
            tc.tile_pool(name="nm", bufs=1) as nm,
            tc.tile_pool(name="ob", bufs=2) as obp,
        ):
            # [partition, chunk, token]: chunks 0-1 = Q^T, 2-3 = K^T
            qkT = pp.tile([128, 4, T], F32R)
            # V1 blocks per (t-chunk, head): even head [V|1@64|0], odd [0|1@32|V@64]
            vsb = pp.tile([128, 16, 4 * 128], F32R)
            yT = pp.tile([128, 2, T], F32R)
            wp = pp.tile([128, 2, D], F32R)
            trisb = pp.tile([128, 128], F32R)
            bqksb = pp.tile([128, 4], F32)
            bvsb = pp.tile([1, HDIM], F32R)
            onesr = pp.tile([1, 128], F32R)

            nc.sync.dma_start(trisb[:], tri[:])
            nc.sync.dma_start(bqksb[:], bqk[:])
            nc.sync.dma_start(bvsb[:], bv[:])
            for c in range(2):
                nc.sync.dma_start(wp[:, c, :], Wp[c * 128 : (c + 1) * 128, :])
            nc.gpsimd.memset(onesr[:].bitcast(F32), 1.0)
            if "A" not in phases:
                nc.gpsimd.memset(qkT[:].bitcast(F32), 0.0)
            if "B" not in phases and "D" in phases:
                nc.gpsimd.memset(yT[:].bitcast(F32), 0.0)
            nc.gpsimd.memset(vsb[:].bitcast(F32), 0.0)
            # ones columns: even heads at 128j+64 (j=0,2), odd at 128j+32 (j=1,3)
            nc.gpsimd.memset(vsb[:, :, 64::256].bitcast(F32), 1.0)
            nc.gpsimd.memset(vsb[:, :, 160::256].bitcast(F32), 1.0)

            loop_ctx = tc.For_i(0, reps, 1) if reps > 1 else contextlib.nullcontext()
            with loop_ctx:
                xs, ws = [], []
                for k in range(KCH):
                    wt = inA.tile([128, 3 * HDIM], F32R, tag=f"w{k}", name=f"wt{k}")
                    if "i" in phases:
                        nc.sync.dma_start(wt[:], Wa[k * 128 : (k + 1) * 128, :])
                    ws.append(wt)
                    xt = inA.tile([128, T], F32R, tag=f"x{k}", name=f"xt{k}")
                    if "i" in phases:
                        nc.sync.dma_start(xt[:], xT[k * 128 : (k + 1) * 128, :])
                    xs.append(xt)

                # ---- wave1 (during DMA): Q/K chunks m=0 (Q) and m=2 (K) for
                # heads 0-1, full T, chunk-gated accumulation over all 8
                # PSUM banks (nothing else runs yet) ----
                def qk_unit(m, sph, pool, tg):
                    acc = pool.tile([128, 1024], F32, tag=tg, name=f"acc{m}{sph}")
                    for k in range(KCH):
                        for h in range(2):
                            nc.tensor.matmul(
                                acc[:, h * 512 : (h + 1) * 512],
                                ws[k][:, m * 128 : (m + 1) * 128],
                                xs[k][:, sph * 1024 + h * 512 : sph * 1024 + (h + 1) * 512],
                                start=(k == 0),
                                stop=(k == KCH - 1),
                            )
                    nc.vector.tensor_scalar_add(
                        qkT[:, m, sph * 1024 : (sph + 1) * 1024],
                        acc[:],
                        bqksb[:, m : m + 1],
                    )

                def qk_wave1():
                    accs = {}
                    for i, m in enumerate([0, 2]):
                        pool = [psS, psO][i]
                        tg = ["s", "o"][i]
                        for sph in range(2):
                            accs[(m, sph)] = pool.tile(
                                [128, 1024], F32, tag=tg, name=f"acc{m}{sph}"
                            )
                    for k in range(KCH):
                        for (m, sph), acc in accs.items():
                            for h in range(2):
                                nc.tensor.matmul(
                                    acc[:, h * 512 : (h + 1) * 512],
                                    ws[k][:, m * 128 : (m + 1) * 128],
                                    xs[k][:, sph * 1024 + h * 512 : sph * 1024 + (h + 1) * 512],
                                    start=(k == 0),
                                    stop=(k == KCH - 1),
                                )
                    for (m, sph), acc in accs.items():
                        nc.vector.tensor_scalar_add(
                            qkT[:, m, sph * 1024 : (sph + 1) * 1024],
                            acc[:],
                            bqksb[:, m : m + 1],
                        )

                if "A" in phases:
                    qk_wave1()

                # ---- filler units: V groups, Q/K wave2 (heads 2-3), proj ----
                def v_group(g):
                    # 4 t-blocks per [128,1024] PSUM tile
                    acc = psO.tile([128, 1024], F32, tag="o", name=f"vacc{g}")
                    for i in range(4):
                        t = 4 * g + i
                        a = acc[:, i * 256 : (i + 1) * 256]
                        ks = [(t + j) % KCH for j in range(KCH)]
                        for j, k in enumerate(ks):
                            nc.tensor.matmul(
                                a,
                                xs[k][:, t * 128 : (t + 1) * 128],
                                ws[k][:, 2 * HDIM : 3 * HDIM],
                                start=(j == 0),
                                stop=False,
                            )
                        nc.tensor.matmul(
                            a,
                            onesr[0:1, 0:128],
                            bvsb[0:1, :],
                            start=False,
                            stop=True,
                        )
                    for i in range(4):
                        t = 4 * g + i
                        a = acc[:, i * 256 : (i + 1) * 256]
                        src_e = a[:, 0:192].rearrange("p (h d) -> p h d", d=64)[:, ::2]
                        dst_e = vsb[:, t, 0:320].rearrange("p (h d) -> p h d", d=64)[:, ::4]
                        nc.vector.tensor_copy(dst_e, src_e)
                        src_o = a[:, 64:256].rearrange("p (h d) -> p h d", d=64)[:, ::2]
                        dst_o = vsb[:, t, 192:512].rearrange("p (h d) -> p h d", d=64)[:, ::4]
                        nc.vector.tensor_copy(dst_o, src_o)

                def w2_unit(m, sph):
                    qk_unit(m, sph, psS, "s")

                def proj_unit(m, sph, eng):
                    acc = psS.tile([128, 1024], F32, tag="s", name=f"pacc{m}{sph}")
                    for c in range(2):
                        for h in range(2):
                            nc.tensor.matmul(
                                acc[:, h * 512 : (h + 1) * 512],
                                wp[:, c, m * 128 : (m + 1) * 128],
                                yT[:, c, sph * 1024 + h * 512 : sph * 1024 + (h + 1) * 512],
                                start=(c == 0),
                                stop=(c == 1),
                            )
                    obt = obp.tile([128, 1024], F32, tag="obt", name="obt")
                    if eng == 0:
                        nc.vector.tensor_copy(obt[:], acc[:])
                    else:
                        nc.scalar.copy(obt[:], acc[:])
                    if "o" in phases:
                        nc.sync.dma_start(
                            out[m * 128 : (m + 1) * 128, sph * 1024 : (sph + 1) * 1024],
                            obt[:],
                        )

                fillers = []
                if "A" in phases:
                    fillers += [
                        lambda: v_group(0),
                        lambda: v_group(1),
                        lambda: w2_unit(1, 0),
                        lambda: w2_unit(3, 0),
                        lambda: v_group(2),
                        lambda: v_group(3),
                        lambda: w2_unit(1, 1),
                        lambda: w2_unit(3, 1),
                    ]

                def pop_filler():
                    if fillers:
                        fillers.pop(0)()

                # ---- attention: half-outer, heads inner, pipelined chain ----
                def head_groups(j, half):
                    h0 = half * HQ
                    ki_hi = 8 if half == 0 else 16
                    return [(j, half, g) for g in _groups(h0, HQ, ki_hi)]

                def emit_S(j, half, ki, grp, masked):
                    # spans are contiguous in q; place span (s, w) at column
                    # s - base so the written region [o, o+total) is gapless
                    po = 64 * (j % 2)
                    qc, kc = j // 2, 2 + j // 2
                    k0 = 128 * ki
                    base = grp[0][0] - grp[0][0] % 512
                    o = grp[0][0] - base
                    ew = grp[-1][0] + grp[-1][1] - base
                    sps = psS.tile([128, 1024], F32, tag="s", name="sps")
                    for s, w in grp:
                        nc.tensor.matmul(
                            sps[:, s - base : s - base + w],
                            qkT[po : po + 64, kc, k0 : k0 + 128],
                            qkT[po : po + 64, qc, s : s + w],
                            start=True,
                            stop=True,
                        )
                    psb = wk.tile([128, 1024], F32R, tag="p", name="psb")
                    nc.scalar.activation(psb[:, o:ew], sps[:, o:ew], EXP, scale=0.125)
                    if masked:
                        nc.vector.tensor_tensor(
                            psb[:, o : o + 128], psb[:, o : o + 128], trisb[:], MULT
                        )
                    return psb, base

                def emit_PV(j, half, ki, grp, psb, base, first, last, oh):
                    h0 = half * HQ
                    for s, w in grp:
                        nc.tensor.matmul(
                            oh[:, s - h0 : s - h0 + w],
                            vsb[:, ki, 128 * j : 128 * (j + 1)],
                            psb[:, s - base : s - base + w],
                            start=first,
                            stop=last,
                            skip_group_check=True,
                        )

                def emit_norm(j, half, oh):
                    po = 64 * (j % 2)
                    qc = j // 2
                    dp = 64 if j % 2 == 0 else 32
                    h0 = half * HQ
                    drec = nm.tile([1, HQ], F32, tag="dc", name="drec")
                    nc.vector.reciprocal(drec[0:1, :], oh[dp : dp + 1, :])
                    rb = nm.tile([128, HQ], F32R, tag="rb", name="rb")
                    nc.gpsimd.partition_broadcast(rb[:], drec[0:1, :].bitcast(F32R))
                    nc.vector.tensor_tensor(
                        yT[po : po + 64, qc, h0 : h0 + HQ],
                        oh[po : po + 64, :],
                        rb[po : po + 64, :],
                        MULT,
                    )

                def attention_half(half, pace):
                    for j in range(HPC):
                        groups = head_groups(j, half)
                        oh = psO.tile([128, HQ], F32, tag="o", name=f"oh{j}{half}")
                        pend = None  # (ki, grp, psb, base, first, last)
                        for gi, (jj, hh, (ki, grp, masked)) in enumerate(groups):
                            psb, base = emit_S(jj, hh, ki, grp, masked)
                            if gi % pace == 0:
                                pop_filler()
                            if pend is not None:
                                emit_PV(j, half, *pend, oh)
                            pend = (ki, grp, psb, base, gi == 0, gi == len(groups) - 1)
                        emit_PV(j, half, *pend, oh)
                        emit_norm(j, half, oh)

                if "B" in phases:
                    attention_half(0, 2)
                while "B" not in phases and fillers and len(fillers) > 8:
                    pop_filler()
                if "D" in phases:
                    fillers += [
                        (lambda m=m: proj_unit(m, 0, m % 2)) for m in range(8)
                    ]
                if "B" in phases:
                    attention_half(1, 8)
                while fillers:
                    pop_filler()
                if "D" in phases:
                    for m in range(8):
                        proj_unit(m, 1, m % 2)
                if "o" in phases and "D" not in phases:
                    for m in range(8):
                        nc.sync.dma_start(
                            out[m * 128 : (m + 1) * 128, :],
                            qkT[:, m % 4, :].bitcast(F32),
                        )
                if "o" not in phases:
                    nc.sync.dma_start(out[0:1, 0:128], qkT[0:1, 0, 0:128].bitcast(F32))

                if debug:
                    nc.sync.dma_start(dbg_qkT[:], qkT[:].bitcast(F32))
                    nc.sync.dma_start(dbg_vsb[:], vsb[:].bitcast(F32))
                    nc.sync.dma_start(dbg_yT[:], yT[:].bitcast(F32))

    nc.compile()
    return nc


def _get_nc():
    global _NC
    if _NC is None:
        _NC = _build()
    return _NC


def _make_in_maps(x, Wqkv, bqkv, Wproj):
    x = np.ascontiguousarray(np.asarray(x, np.float32))
    Wqkv = np.asarray(Wqkv, np.float32)
    bqkv = np.asarray(bqkv, np.float32)
    Wproj = np.asarray(Wproj, np.float32)
    tri = np.triu(np.ones((128, 128), np.float32))  # tri[k, q] = q >= k
    in_maps = []
    for c in range(NCORES):
        b = c // 4
        cs = (c % 4) * HDIM
        ce = cs + HDIM
        Wa = np.concatenate(
            [Wqkv[:, cs:ce], Wqkv[:, D + cs : D + ce], Wqkv[:, 2 * D + cs : 2 * D + ce]],
            axis=1,
        )
        bqk_c = np.concatenate([bqkv[cs:ce], bqkv[D + cs : D + ce]])
        in_maps.append(
            {
                "xT": np.ascontiguousarray(x[b].T),
                "Wa": np.ascontiguousarray(Wa),
                "Wp": np.ascontiguousarray(Wproj[cs:ce, :]),
                "bqk": np.ascontiguousarray(bqk_c.reshape(4, 128).T),
                "bv": np.ascontiguousarray(bqkv[2 * D + cs : 2 * D + ce].reshape(1, HDIM)),
                "tri": tri,
            }
        )
    return in_maps


def _run(in_maps, **kwargs):
    nc = _get_nc()
    return run_bass_kernel_spmd(nc, in_maps, core_ids=list(range(NCORES)), **kwargs)


def kernel(x, Wqkv, bqkv, Wproj, bproj):
    in_maps = _make_in_maps(x, Wqkv, bqkv, Wproj)
    res = _run(in_maps)
    bproj = np.asarray(bproj, np.float32)
    outp = np.zeros((B, T, D), np.float64)
    for c in range(NCORES):
        outp[c // 4] += res.results[c]["out"].T.astype(np.float64)
    outp += bproj[None, None, :].astype(np.float64)
    return outp.astype(np.float32)


# revision 47
# speedup vs baseline: 1.7270x; 1.0018x over previous
"""Causal self-attention (B=2, T=2048, D=1024, NH=16) on 8 Trainium2 NeuronCores.

Sharding: core c handles batch b = c // 4 and heads [4*(c%4), 4*(c%4)+4)
(data parallel over B, head/tensor parallel over NH). Wqkv is column-sliced,
Wproj row-sliced per core; each core emits its partial projection
out_c = Y_c @ Wproj_c (transposed, [D, T]); the host sums the four partials
per batch and adds bproj. No cross-core collectives are needed.

Schedule (HW-calibrated): the ACT engine's exp (~750ns/instr + 0.34ns/col)
and the PE matmul stream are the joint bottleneck, so the kernel is built
as one long software-pipelined stream:
  1. During the input DMA (~37us), Q/K for heads 0-1 (chunks m=0,2) are
     accumulated chunk-by-chunk over the full sequence (4x PSUM [128,2048]
     spread over 2 pools = all 8 banks).
  2. Attention runs half-outer (q in [0,1024) for all heads, then
     [1024,2048)), heads inner. Within a head the S->exp->(tri)->PV chain
     is pipelined with depth 2: PV(g-1) issues after S(g), so the exp of
     group g-1 hides under the S matmul of group g.
  3. Remaining PE work (Q/K for heads 2-3, the V projection, and the
     output projection) is emitted as filler units between attention
     groups: attention is ACT-paced, so fillers use PE idle slots.
  4. The output projection is split per q-half: half-0 projection+DMA
     overlap half-1 attention.

Matmuls run in float32r; all per-(head,half,group) math is bit-identical
to the original reference kernel (same span/group structure, same
accumulation orders).
"""

import contextlib

import numpy as np

import concourse.bass as bass
import concourse.mybir as mybir
import concourse.tile as tile
from concourse import bacc
from concourse.bass_utils import run_bass_kernel_spmd

B, T, D = 2, 2048, 1024
NH, HD = 16, 64
NCORES = 8
HPC = 4                 # heads per core
HDIM = HPC * HD         # 256 qkv dims per core
KCH = D // 128          # 8 contraction chunks
HQ = T // 2             # q-half length (1024)
F32 = mybir.dt.float32
F32R = mybir.dt.float32r
EXP = mybir.ActivationFunctionType.Exp
MULT = mybir.AluOpType.mult

_NC = None


def _spans(lo, hi):
    """q-spans [s, s+w) covering [lo, hi), each ending on a 512 boundary."""
    out = []
    s = lo
    while s < hi:
        w = min(512 - (s % 512), hi - s)
        out.append((s, w))
        s += w
    return out


def _groups(h0, hq, ki_hi):
    """(ki, grp, masked) list for one q-half: grp = 1-2 spans sharing one
    PSUM tile + one exp; masked = group starts at the diagonal block."""
    out = []
    for ki in range(ki_hi):
        k0 = 128 * ki
        spl = _spans(max(k0, h0), h0 + hq)
        for i in range(0, len(spl), 2):
            grp = spl[i : i + 2]
            out.append((ki, grp, grp[0][0] == k0))
    return out


def _build(debug=False, reps=1, phases="iABDo"):
    """phases (ablation/bench only): i=input DMA, A=QKV matmuls,
    B=attention, D=proj matmuls, o=output DMA. Default = full kernel."""
    nc = bacc.Bacc()
    xT = nc.declare_dram_parameter("xT", [D, T], F32R, isOutput=False)
    Wa = nc.declare_dram_parameter("Wa", [D, 3 * HDIM], F32R, isOutput=False)
    Wp = nc.declare_dram_parameter("Wp", [HDIM, D], F32R, isOutput=False)
    bqk = nc.declare_dram_parameter("bqk", [128, 4], F32, isOutput=False)
    bv = nc.declare_dram_parameter("bv", [1, HDIM], F32R, isOutput=False)
    tri = nc.declare_dram_parameter("tri", [128, 128], F32R, isOutput=False)
    out = nc.declare_dram_parameter("out", [D, T], F32, isOutput=True)
    if debug:
        dbg_qkT = nc.declare_dram_parameter("dbg_qkT", [128, 4 * T], F32, isOutput=True)
        dbg_vsb = nc.declare_dram_parameter("dbg_vsb", [128, 16 * 512], F32, isOutput=True)
        dbg_yT = nc.declare_dram_parameter("dbg_yT", [128, 2 * T], F32, isOutput=True)

    with tile.TileContext(nc) as tc:
        with (
            tc.tile_pool(name="persist", bufs=1) as pp,
            tc.tile_pool(name="psS", bufs=2, space="PSUM") as psS,
            tc.tile_pool(name="psO", bufs=2, space="PSUM") as psO,
            tc.tile_pool(name="inA", bufs=1) as inA,
            tc.tile_pool(name="wk", bufs=2) as wk,
            tc.tile_pool(name="nm", bufs=1) as nm,
            tc.tile_pool(name="ob", bufs=2) as obp,
        ):
            # [partition, chunk, token]: chunks 0-1 = Q^T, 2-3 = K^T
            qkT = pp.tile([128, 4, T], F32R)
            # V1 blocks per (t-chunk, head): even head [V|1@64|0], odd [0|1@32|V@64]
            vsb = pp.tile([128, 16, 4 * 128], F32R)
            yT = pp.tile([128, 2, T], F32R)
            wp = pp.tile([128, 2, D], F32R)
            trisb = pp.tile([128, 128], F32R)
            bqksb = pp.tile([128, 4], F32)
            bvsb = pp.tile([1, HDIM], F32R)
            onesr = pp.tile([1, 128], F32R)

            nc.sync.dma_start(trisb[:], tri[:])
            nc.sync.dma_start(bqksb[:], bqk[:])
            nc.sync.dma_start(bvsb[:], bv[:])
            for c in range(2):
                nc.sync.dma_start(wp[:, c, :], Wp[c * 128 : (c + 1) * 128, :])
            nc.gpsimd.memset(onesr[:].bitcast(F32), 1.0)
            if "A" not in phases:
                nc.gpsimd.memset(qkT[:].bitcast(F32), 0.0)
            if "B" not in phases and "D" in phases:
                nc.gpsimd.memset(yT[:].bitcast(F32), 0.0)
            nc.gpsimd.memset(vsb[:].bitcast(F32), 0.0)
            # ones columns: even heads at 128j+64 (j=0,2), odd at 128j+32 (j=1,3)
            nc.gpsimd.memset(vsb[:, :, 64::256].bitcast(F32), 1.0)
            nc.gpsimd.memset(vsb[:, :, 160::256].bitcast(F32), 1.0)

            loop_ctx = tc.For_i(0, reps, 1) if reps > 1 else contextlib.nullcontext()
            with loop_ctx:
                xs, ws = [], []
                for k in range(KCH):
                    wt = inA.tile([128, 3 * HDIM], F32R, tag=f"w{k}", name=f"wt{k}")
                    if "i" in phases:
                        nc.sync.dma_start(wt[:], Wa[k * 128 : (k + 1) * 128, :])
                    ws.append(wt)
                    xt = inA.tile([128, T], F32R, tag=f"x{k}", name=f"xt{k}")
                    if "i" in phases:
                        nc.sync.dma_start(xt[:], xT[k * 128 : (k + 1) * 128, :])
                    xs.append(xt)

                # ---- wave1 (during DMA): Q/K chunks m=0 (Q) and m=2 (K) for
                # heads 0-1, full T, chunk-gated accumulation over all 8
                # PSUM banks (nothing else runs yet) ----
                def qk_unit(m, sph, pool, tg):
                    acc = pool.tile([128, 1024], F32, tag=tg, name=f"acc{m}{sph}")
                    for k in range(KCH):
                        for h in range(2):
                            nc.tensor.matmul(
                                acc[:, h * 512 : (h + 1) * 512],
                                ws[k][:, m * 128 : (m + 1) * 128],
                                xs[k][:, sph * 1024 + h * 512 : sph * 1024 + (h + 1) * 512],
                                start=(k == 0),
                                stop=(k == KCH - 1),
                            )
                    nc.vector.tensor_scalar_add(
                        qkT[:, m, sph * 1024 : (sph + 1) * 1024],
                        acc[:],
                        bqksb[:, m : m + 1],
                    )

                def qk_wave1():
                    accs = {}
                    for i, m in enumerate([0, 2]):
                        pool = [psS, psO][i]
                        tg = ["s", "o"][i]
                        for sph in range(2):
                            accs[(m, sph)] = pool.tile(
                                [128, 1024], F32, tag=tg, name=f"acc{m}{sph}"
                            )
                    for k in range(KCH):
                        for (m, sph), acc in accs.items():
                            for h in range(2):
                                nc.tensor.matmul(
                                    acc[:, h * 512 : (h + 1) * 512],
                                    ws[k][:, m * 128 : (m + 1) * 128],
                                    xs[k][:, sph * 1024 + h * 512 : sph * 1024 + (h + 1) * 512],
                                    start=(k == 0),
                                    stop=(k == KCH - 1),
                                )
                    for (m, sph), acc in accs.items():
                        nc.vector.tensor_scalar_add(
                            qkT[:, m, sph * 1024 : (sph + 1) * 1024],
                            acc[:],
                            bqksb[:, m : m + 1],
                        )

                if "A" in phases:
                    qk_wave1()

                # ---- filler units: V groups, Q/K wave2 (heads 2-3), proj ----
                def v_group(g):
                    # 4 t-blocks per [128,1024] PSUM tile
                    acc = psO.tile([128, 1024], F32, tag="o", name=f"vacc{g}")
                    for i in range(4):
                        t = 4 * g + i
                        a = acc[:, i * 256 : (i + 1) * 256]
                        ks = [(t + j) % KCH for j in range(KCH)]
                        for j, k in enumerate(ks):
                            nc.tensor.matmul(
                                a,
                                xs[k][:, t * 128 : (t + 1) * 128],
                                ws[k][:, 2 * HDIM : 3 * HDIM],
                                start=(j == 0),
                                stop=False,
                            )
                        nc.tensor.matmul(
                            a,
                            onesr[0:1, 0:128],
                            bvsb[0:1, :],
                            start=False,
                            stop=True,
                        )
                    for i in range(4):
                        t = 4 * g + i
                        a = acc[:, i * 256 : (i + 1) * 256]
                        src_e = a[:, 0:192].rearrange("p (h d) -> p h d", d=64)[:, ::2]
                        dst_e = vsb[:, t, 0:320].rearrange("p (h d) -> p h d", d=64)[:, ::4]
                        nc.vector.tensor_copy(dst_e, src_e)
                        src_o = a[:, 64:256].rearrange("p (h d) -> p h d", d=64)[:, ::2]
                        dst_o = vsb[:, t, 192:512].rearrange("p (h d) -> p h d", d=64)[:, ::4]
                        nc.vector.tensor_copy(dst_o, src_o)

                def w2_unit(m, sph):
                    qk_unit(m, sph, psS, "s")

                def proj_unit(m, sph, eng):
                    acc = psS.tile([128, 1024], F32, tag="s", name=f"pacc{m}{sph}")
                    for c in range(2):
                        for h in range(2):
                            nc.tensor.matmul(
                                acc[:, h * 512 : (h + 1) * 512],
                                wp[:, c, m * 128 : (m + 1) * 128],
                                yT[:, c, sph * 1024 + h * 512 : sph * 1024 + (h + 1) * 512],
                                start=(c == 0),
                                stop=(c == 1),
                            )
                    obt = obp.tile([128, 1024], F32, tag="obt", name="obt")
                    if eng == 0:
                        nc.vector.tensor_copy(obt[:], acc[:])
                    else:
                        nc.scalar.copy(obt[:], acc[:])
                    if "o" in phases:
                        nc.sync.dma_start(
                            out[m * 128 : (m + 1) * 128, sph * 1024 : (sph + 1) * 1024],
                            obt[:],
                        )

                fillers = []
                if "A" in phases:
                    fillers += [
                        lambda: v_group(0),
                        lambda: v_group(1),
                        lambda: w2_unit(1, 0),
                        lambda: w2_unit(3, 0),
                        lambda: v_group(2),
                        lambda: v_group(3),
                        lambda: w2_unit(1, 1),
                        lambda: w2_unit(3, 1),
                    ]

                def pop_filler():
                    if fillers:
                        fillers.pop(0)()

                # ---- attention: half-outer, heads inner, pipelined chain ----
                def head_groups(j, half):
                    h0 = half * HQ
                    ki_hi = 8 if half == 0 else 16
                    return [(j, half, g) for g in _groups(h0, HQ, ki_hi)]

                def emit_S(j, half, ki, grp, masked):
                    # spans are contiguous in q; place span (s, w) at column
                    # s - base so the written region [o, o+total) is gapless
                    po = 64 * (j % 2)
                    qc, kc = j // 2, 2 + j // 2
                    k0 = 128 * ki
                    base = grp[0][0] - grp[0][0] % 512
                    o = grp[0][0] - base
                    ew = grp[-1][0] + grp[-1][1] - base
                    sps = psS.tile([128, 1024], F32, tag="s", name="sps")
                    for s, w in grp:
                        nc.tensor.matmul(
                            sps[:, s - base : s - base + w],
                            qkT[po : po + 64, kc, k0 : k0 + 128],
                            qkT[po : po + 64, qc, s : s + w],
                            start=True,
                            stop=True,
                        )
                    psb = wk.tile([128, 1024], F32R, tag="p", name="psb")
                    nc.scalar.activation(psb[:, o:ew], sps[:, o:ew], EXP, scale=0.125)
                    if masked:
                        nc.vector.tensor_tensor(
                            psb[:, o : o + 128], psb[:, o : o + 128], trisb[:], MULT
                        )
                    return psb, base

                def emit_PV(j, half, ki, grp, psb, base, first, last, oh):
                    h0 = half * HQ
                    for s, w in grp:
                        nc.tensor.matmul(
                            oh[:, s - h0 : s - h0 + w],
                            vsb[:, ki, 128 * j : 128 * (j + 1)],
                            psb[:, s - base : s - base + w],
                            start=first,
                            stop=last,
                            skip_group_check=True,
                        )

                def emit_norm(j, half, oh):
                    po = 64 * (j % 2)
                    qc = j // 2
                    dp = 64 if j % 2 == 0 else 32
                    h0 = half * HQ
                    drec = nm.tile([1, HQ], F32, tag="dc", name="drec")
                    nc.vector.reciprocal(drec[0:1, :], oh[dp : dp + 1, :])
                    rb = nm.tile([128, HQ], F32R, tag="rb", name="rb")
                    nc.gpsimd.partition_broadcast(rb[:], drec[0:1, :].bitcast(F32R))
                    nc.vector.tensor_tensor(
                        yT[po : po + 64, qc, h0 : h0 + HQ],
                        oh[po : po + 64, :],
                        rb[po : po + 64, :],
                        MULT,
                    )

                def attention_half(half, pace):
                    for j in range(HPC):
                        groups = head_groups(j, half)
                        oh = psO.tile([128, HQ], F32, tag="o", name=f"oh{j}{half}")
                        pend = None  # (ki, grp, psb, base, first, last)
                        for gi, (jj, hh, (ki, grp, masked)) in enumerate(groups):
                            psb, base = emit_S(jj, hh, ki, grp, masked)
                            if gi % pace == 0:
                                pop_filler()
                            if pend is not None:
                                emit_PV(j, half, *pend, oh)
                            pend = (ki, grp, psb, base, gi == 0, gi == len(groups) - 1)
                        emit_PV(j, half, *pend, oh)
                        emit_norm(j, half, oh)

                if "B" in phases:
                    attention_half(0, 2)
                while "B" not in phases and fillers and len(fillers) > 8:
                    pop_filler()
                if "D" in phases:
                    fillers += [
                        (lambda m=m: proj_unit(m, 0, m % 2)) for m in range(8)
                    ]
                if "B" in phases:
                    attention_half(1, 8)
                while fillers:
                    pop_filler()
                if "D" in phases:
                    for m in range(8):
                        proj_unit(m, 1, m % 2)
                if "o" in phases and "D" not in phases:
                    for m in range(8):
                        nc.sync.dma_start(
                            out[m * 128 : (m + 1) * 128, :],
                            qkT[:, m % 4, :].bitcast(F32),
                        )
                if "o" not in phases:
                    nc.sync.dma_start(out[0:1, 0:128], qkT[0:1, 0, 0:128].bitcast(F32))

                if debug:
                    nc.sync.dma_start(dbg_qkT[:], qkT[:].bitcast(F32))
                    nc.sync.dma_start(dbg_vsb[:], vsb[:].bitcast(F32))
                    nc.sync.dma_start(dbg_yT[:], yT[:].bitcast(F32))

    nc.compile()
    return nc


def _get_nc():
    global _NC
    if _NC is None:
        _NC = _build()
    return _NC


def _make_in_maps(x, Wqkv, bqkv, Wproj):
    x = np.ascontiguousarray(np.asarray(x, np.float32))
    Wqkv = np.asarray(Wqkv, np.float32)
    bqkv = np.asarray(bqkv, np.float32)
    Wproj = np.asarray(Wproj, np.float32)
    tri = np.triu(np.ones((128, 128), np.float32))  # tri[k, q] = q >= k
    in_maps = []
    for c in range(NCORES):
        b = c // 4
        cs = (c % 4) * HDIM
        ce = cs + HDIM
        Wa = np.concatenate(
            [Wqkv[:, cs:ce], Wqkv[:, D + cs : D + ce], Wqkv[:, 2 * D + cs : 2 * D + ce]],
            axis=1,
        )
        bqk_c = np.concatenate([bqkv[cs:ce], bqkv[D + cs : D + ce]])
        in_maps.append(
            {
                "xT": np.ascontiguousarray(x[b].T),
                "Wa": np.ascontiguousarray(Wa),
                "Wp": np.ascontiguousarray(Wproj[cs:ce, :]),
                "bqk": np.ascontiguousarray(bqk_c.reshape(4, 128).T),
                "bv": np.ascontiguousarray(bqkv[2 * D + cs : 2 * D + ce].reshape(1, HDIM)),
                "tri": tri,
            }
        )
    return in_maps


def _run(in_maps, **kwargs):
    nc = _get_nc()
    return run_bass_kernel_spmd(nc, in_maps, core_ids=list(range(NCORES)), **kwargs)


def kernel(x, Wqkv, bqkv, Wproj, bproj):
    in_maps = _make_in_maps(x, Wqkv, bqkv, Wproj)
    res = _run(in_maps)
    bproj = np.asarray(bproj, np.float32)
    outp = np.zeros((B, T, D), np.float64)
    for c in range(NCORES):
        outp[c // 4] += res.results[c]["out"].T.astype(np.float64)
    outp += bproj[None, None, :].astype(np.float64)
    return outp.astype(np.float32)
